# revision 1
# baseline (speedup 1.0000x reference)
"""Trainium2 Bass kernel for nn_Block_57921928954115 (dense transformer block).

Sharding: 8 cores = 4 batches x 2 token-half-shards. Core (b, 0) owns token
chunks {0:256, 768:1024} of batch b; core (b, 1) owns {256:512, 512:768}
(balanced causal load). K/V are computed redundantly within each pair so
attention needs no collectives. The two global (whole-tensor) RMS norms each
use one scalar AllReduce across the 8 cores, algebraically deferred so
matmuls overlap them: s1^2 folds into the q-side rope tables, s1 into the
softmax-denominator reciprocal, s2 into the u eviction.

Activations live in transposed [feature, token] layout on chip; weights are
consumed as [K, N] lhsT tiles straight from HBM in float32r (full PE rate,
~1.4e-4 matmul rel err measured on HW). Softmax denominators come free via a
65th ones-column on the V stationary operand. Causality is enforced with
additive -1e30 mask inputs; chunk-lo is padded to kv 512, chunk-hi to 1024,
which keeps the program identical across cores (SPMD).
"""
import numpy as np

B, T, C, H, S = 4, 1024, 768, 12, 64
FF, HID = 3072, 2048
EPS = 1e-6
NC = 8
TOK = 512          # own tokens per core
CHK = 256          # q chunk
NUMEL = float(B * T * C)
CT, FT, HT = C // 128, FF // 128, HID // 128
TT8, QT = T // 128, TOK // 128

_cache = {}


def _rope_tables_np():
    theta = (10000.0 ** (-2.0 * np.arange(0, S, 2, dtype=np.float32) / S))
    theta = theta.astype(np.float32)
    freqs = np.arange(T, dtype=np.float32)[:, None] * theta[None, :]
    cos = np.repeat(np.cos(freqs), 2, axis=-1).astype(np.float32)  # [T,64]
    sin = np.repeat(np.sin(freqs), 2, axis=-1).astype(np.float32)
    return cos, sin


def _chunks_for(half):
    if half == 0:
        return [(0, 256), (768, 1024)]
    return [(256, 512), (512, 768)]


def _own_idx(half):
    (a0, a1), (b0, b1) = _chunks_for(half)
    return np.concatenate([np.arange(a0, a1), np.arange(b0, b1)])


class _Stop(Exception):
    pass


def _build_program(collectives=True, stop_after=99, reps=1,
                   ff_bf16=False):
    import concourse.mybir as mybir
    import concourse.tile as tile
    from concourse import bacc
    from concourse.masks import make_identity

    F32 = mybir.dt.float32
    F32R = mybir.dt.float32r
    BF16 = mybir.dt.bfloat16
    FDT = BF16 if ff_bf16 else F32R
    AX = mybir.AxisListType.X
    ALU = mybir.AluOpType
    AF = mybir.ActivationFunctionType

    nc = bacc.Bacc("TRN2", target_bir_lowering=False, debug=False,
                   enable_asserts=True, num_devices=NC)

    def din(name, shape, dt=F32):
        return nc.dram_tensor(name, list(shape), dt, kind="ExternalInput")

    xb_d = din("xb", [T, C])
    xq_d = din("xq", [TOK, C])
    wq_d = din("wq", [C, C], F32R)
    wk_d = din("wk", [C, C], F32R)
    wv_d = din("wv", [C, C], F32R)
    wo_d = din("wo", [C, C], F32R)
    w1_d = din("w1", [C, FF], F32R)
    ww_d = din("ww", [FF, HID], FDT)
    wg_d = din("wg", [FF, HID], FDT)
    wd_d = din("wd", [HID, FF], FDT)
    w2_d = din("w2", [FF, C], F32R)
    g1_d = din("g1c", [128, CT])
    g2_d = din("g2c", [128, CT])
    b1_d = din("b1c", [128, FT])
    bw_d = din("bwc", [128, HT])
    bg_d = din("bgc", [128, HT])
    bd_d = din("bdc", [128, FT])
    b2_d = din("b2c", [128, CT])
    cos_d = din("cosf", [128, T])
    sin_d = din("sinf", [128, T])
    cosq_d = din("cosq", [128, TOK])
    sinq_d = din("sinq", [128, TOK])
    rmat_d = din("rmat", [128, 128], F32R)
    mlo_d = din("mlo", [512, CHK])     # additive mask^T chunk-lo [tk, tq]
    mhi_d = din("mhi", [T, CHK])       # additive mask^T chunk-hi
    y_d = nc.dram_tensor("yT", [C, TOK], F32, kind="ExternalOutput")

    with tile.TileContext(nc) as tc:
        from contextlib import ExitStack
        for _rep in range(reps):
            es = ExitStack()
            _open = []

            def _new(**kw):
                p = tc.alloc_tile_pool(**kw)
                _open.append(p)
                return p

            def _rel(p):
                p.release()
                _open.remove(p)

            def _cut(n):
                if stop_after <= n:
                    raise _Stop()
            const = es.enter_context(tc.tile_pool(name="const", bufs=1))

            # ---------------- constants ----------------
            ident = const.tile([128, 128], F32, name="ident")
            make_identity(nc, ident[:])
            ones_col = const.tile([128, 1], F32, name="ones_col")
            nc.vector.memset(ones_col[:], 1.0)
            onesH = const.tile([128, H], F32, name="onesH")
            nc.vector.memset(onesH[:], 1.0)
            rmat = const.tile([128, 128], F32R, name="rmat")
            nc.sync.dma_start(rmat[:], rmat_d.ap())
            cosf = const.tile([128, T], F32, name="cosf")
            sinf = const.tile([128, T], F32, name="sinf")
            cosq0 = const.tile([128, TOK], F32, name="cosq0")
            sinq0 = const.tile([128, TOK], F32, name="sinq0")
            for t_, d_ in ((cosf, cos_d), (sinf, sin_d), (cosq0, cosq_d),
                           (sinq0, sinq_d)):
                nc.sync.dma_start(t_[:], d_.ap())
            # masks stored [128, ntiles*CHK]: tk-tile kt at cols kt*CHK:(kt+1)*CHK
            mlo = const.tile([128, 4 * CHK], F32, name="mlo")
            mhi = const.tile([128, 8 * CHK], F32, name="mhi")
            nc.sync.dma_start(
                mlo[:].rearrange("p (a q) -> p a q", q=CHK),
                mlo_d.ap().rearrange("(a p) q -> p a q", p=128))
            nc.sync.dma_start(
                mhi[:].rearrange("p (a q) -> p a q", q=CHK),
                mhi_d.ap().rearrange("(a p) q -> p a q", p=128))
            g1c = const.tile([128, CT], F32, name="g1c")
            g2c = const.tile([128, CT], F32, name="g2c")
            b1c = const.tile([128, FT], F32, name="b1c")
            bwc = const.tile([128, HT], F32, name="bwc")
            bgc = const.tile([128, HT], F32, name="bgc")
            bdc = const.tile([128, FT], F32, name="bdc")
            b2c = const.tile([128, CT], F32, name="b2c")
            for t_, d_ in ((g1c, g1_d), (g2c, g2_d), (b1c, b1_d), (bwc, bw_d),
                           (bgc, bg_d), (bdc, bd_d), (b2c, b2_d)):
                nc.sync.dma_start(t_[:], d_.ap())
            sca = const.tile([1, 8], F32, name="sca")      # scalar scratch 1
            scb = const.tile([1, 8], F32, name="scb")      # scalar scratch 2
            s2c_b = const.tile([128, 1], F32, name="s2c_b")
            s2b = const.tile([128, 1], F32, name="s2b")
            cosq = const.tile([128, TOK], F32, name="cosq")
            sinq = const.tile([128, TOK], F32, name="sinq")
            ss_sb = const.tile([128, 8], F32, name="ss_sb")
            ss2_sb = const.tile([128, 8], F32, name="ss2_sb")

            dram = es.enter_context(tc.tile_pool(name="dram", bufs=1,
                                                 space="DRAM"))
            ar1_in = dram.tile([1, 1], F32, name="ar1_in")
            ar1_out = dram.tile([1, 1], F32, name="ar1_out")
            ar2_in = dram.tile([1, 1], F32, name="ar2_in")
            ar2_out = dram.tile([1, 1], F32, name="ar2_out")

            try:
                # left-stack pools, reserved in lifetime order (LIFO discipline)
                p_xq = es.enter_context(tc.tile_pool(name="p_xq", bufs=1))
                p_qk = _new(name="p_qk", bufs=1)
                p_kv = _new(name="p_kv", bufs=1)
                p_x = _new(name="p_x", bufs=1)
                xT = [p_x.tile([128, T], F32R, name=f"xT{i}") for i in range(CT)]
                xqT = [p_xq.tile([128, TOK], F32R, name=f"xqT{i}") for i in range(CT)]
                qsb = [p_qk.tile([128, TOK], F32R, name=f"qsb{i}") for i in range(CT)]
                ksb = [p_qk.tile([128, T], F32R, name=f"ksb{i}") for i in range(CT)]
                vsb = [p_kv.tile([128, H * 65], F32R, name=f"vsb{i}")
                       for i in range(TT8)]

                # ---------------- x transposes ----------------
                with tc.tile_pool(name="xin", bufs=3) as xin, \
                     tc.tile_pool(name="tps", bufs=4, space="PSUM") as tps:
                    for tt in range(TT8):
                        pan = xin.tile([128, C], F32, name="pan", tag="pan")
                        nc.sync.dma_start(pan[:], xb_d.ap()[tt * 128:(tt + 1) * 128, :])
                        for ct in range(CT):
                            pt = tps.tile([128, 128], F32, name="tp", tag="tp")
                            nc.tensor.transpose(
                                pt[:], pan[:, ct * 128:(ct + 1) * 128], ident[:])
                            nc.vector.tensor_copy(
                                xT[ct][:, tt * 128:(tt + 1) * 128], pt[:])
                    for tt in range(QT):
                        pan = xin.tile([128, C], F32, name="qpan", tag="pan")
                        nc.sync.dma_start(pan[:], xq_d.ap()[tt * 128:(tt + 1) * 128, :])
                        for ct in range(CT):
                            pt = tps.tile([128, 128], F32, name="qtp", tag="tp")
                            nc.tensor.transpose(
                                pt[:], pan[:, ct * 128:(ct + 1) * 128], ident[:])
                            nc.vector.tensor_copy(
                                xqT[ct][:, tt * 128:(tt + 1) * 128], pt[:])

                # ---------------- sum(x^2) -> AR1 ----------------
                s1ps = _new(name="s1ps", bufs=1, space="PSUM")
                with tc.tile_pool(name="sqp", bufs=2) as sqp:
                    for ct in range(CT):
                        sq = sqp.tile([128, TOK], F32, name="sq", tag="sq")
                        nc.vector.tensor_tensor(sq[:], xqT[ct][:], xqT[ct][:],
                                                op=ALU.mult)
                        nc.vector.reduce_sum(ss_sb[:, ct:ct + 1], sq[:], axis=AX)
                nc.vector.tensor_tensor(ss_sb[:, 6:7], ss_sb[:, 0:1], ss_sb[:, 1:2],
                                        op=ALU.add)
                for ct in range(2, CT):
                    nc.vector.tensor_tensor(ss_sb[:, 6:7], ss_sb[:, 6:7],
                                            ss_sb[:, ct:ct + 1], op=ALU.add)
                ssp = s1ps.tile([1, 1], F32, name="ssp", tag="ss")
                nc.tensor.matmul(ssp[:], lhsT=ss_sb[:, 6:7], rhs=ones_col[:],
                                 start=True, stop=True)
                nc.vector.tensor_copy(sca[:, 7:8], ssp[:])
                nc.sync.dma_start(ar1_in[:], sca[:, 7:8])
                if collectives:
                    nc.gpsimd.collective_compute(
                        "AllReduce", ALU.add, replica_groups=[list(range(NC))],
                        ins=[ar1_in.opt()], outs=[ar1_out.opt()])
                else:
                    nc.sync.dma_start(ar1_out[:], ar1_in[:])
                _cut(1)

                # ---------------- QKV (overlaps AR1) ----------------
                mmp = _new(name="mmp", bufs=7, space="PSUM",
                                         side="right")
                wqk = _new(name="wqk", bufs=4)

                for (w_d, dst, srcl, ntok) in ((wq_d, qsb, xqT, TOK),
                                               (wk_d, ksb, xT, T)):
                    for nn in range(ntok // 512):
                        for grp in range(2):
                            pts = [mmp.tile([128, 512], F32, name="qkps", tag="mm")
                                   for _ in range(3)]
                            for k in range(CT):
                                wt = wqk.tile([128, 384], F32R, name="wqkt",
                                              tag="wqkt")
                                nc.sync.dma_start(
                                    wt[:], w_d.ap()[k * 128:(k + 1) * 128,
                                                    grp * 384:(grp + 1) * 384])
                                nc.vector.tensor_scalar(
                                    wt[:], wt[:], g1c[:, k:k + 1], None, op0=ALU.mult)
                                for mi in range(3):
                                    nc.tensor.matmul(
                                        pts[mi][:],
                                        lhsT=wt[:, mi * 128:(mi + 1) * 128],
                                        rhs=srcl[k][:, nn * 512:(nn + 1) * 512],
                                        start=(k == 0), stop=(k == CT - 1))
                            for mi in range(3):
                                m = grp * 3 + mi
                                nc.scalar.copy(dst[m][:, nn * 512:(nn + 1) * 512],
                                               pts[mi][:])

                wvt = [wqk.tile([128, C], F32R, name=f"wvt{k}") for k in range(CT)]
                for k in range(CT):
                    nc.sync.dma_start(wvt[k][:], wv_d.ap()[k * 128:(k + 1) * 128, :])
                    nc.vector.tensor_scalar(wvt[k][:], wvt[k][:], g1c[:, k:k + 1],
                                            None, op0=ALU.mult)
                for tt in range(TT8):
                    for nn in range(2):
                        pt = mmp.tile([128, 384], F32, name="vps", tag="mm")
                        for k in range(CT):
                            nc.tensor.matmul(
                                pt[:], lhsT=xT[k][:, tt * 128:(tt + 1) * 128],
                                rhs=wvt[k][:, nn * 384:(nn + 1) * 384],
                                start=(k == 0), stop=(k == CT - 1))
                        nc.vector.tensor_copy(
                            vsb[tt][:].rearrange("p (h s) -> p h s", s=65)
                            [:, nn * 6:(nn + 1) * 6, 0:64],
                            pt[:].rearrange("p (h s) -> p h s", s=64))
                    nc.vector.tensor_copy(
                        vsb[tt][:].rearrange("p (h s) -> p h s", s=65)
                        [:, :, 64:65].squeeze(),
                        onesH[:])
                _cut(2)

                _rel(wqk)
                _rel(p_x)        # xT dead after k/v matmuls

                # ---------------- rope (in place: ksb->rk, qsb->rq) ----------------
                rqT, rkT = qsb, ksb
                with tc.tile_pool(name="ropet", bufs=3) as rtmp:

                    def rope(dst_tiles, src_tiles, cos_t, sin_t, ntok):
                        for m in range(CT):
                            for nn in range(ntok // 512):
                                sl = slice(nn * 512, (nn + 1) * 512)
                                pt = mmp.tile([128, 512], F32, name="rops", tag="mm")
                                nc.tensor.matmul(pt[:], lhsT=rmat[:],
                                                 rhs=src_tiles[m][:, sl],
                                                 start=True, stop=True)
                                t1 = rtmp.tile([128, 512], F32, name="rt1", tag="rt")
                                nc.vector.tensor_tensor(t1[:], pt[:], sin_t[:, sl],
                                                        op=ALU.mult)
                                t2 = rtmp.tile([128, 512], F32, name="rt2", tag="rt")
                                nc.vector.tensor_tensor(t2[:], src_tiles[m][:, sl],
                                                        cos_t[:, sl], op=ALU.mult)
                                nc.vector.tensor_tensor(dst_tiles[m][:, sl],
                                                        t1[:], t2[:], op=ALU.add)

                    rope(rkT, ksb, cosf, sinf, T)

                    # AR1 -> s1 = 1/sqrt(eps+mean); scale q-side tables
                    nc.sync.dma_start(sca[:, 0:1], ar1_out[:])
                    nc.vector.tensor_scalar(sca[:, 1:2], sca[:, 0:1], 1.0 / NUMEL,
                                            EPS, op0=ALU.mult, op1=ALU.add)
                    nc.scalar.sqrt(sca[:, 2:3], sca[:, 1:2])
                    nc.vector.reciprocal(sca[:, 3:4], sca[:, 2:3])     # s1
                    nc.vector.tensor_tensor(sca[:, 4:5], sca[:, 3:4], sca[:, 3:4],
                                            op=ALU.mult)               # s1^2
                    nc.vector.tensor_scalar(sca[:, 5:6], sca[:, 4:5],
                                            float(C) ** -0.5, None, op0=ALU.mult)
                    nc.gpsimd.partition_broadcast(s2c_b[:], sca[:, 5:6])
                    nc.vector.tensor_scalar(cosq[:], cosq0[:], s2c_b[:], None,
                                            op0=ALU.mult)
                    nc.vector.tensor_scalar(sinq[:], sinq0[:], s2c_b[:], None,
                                            op0=ALU.mult)
                    rope(rqT, qsb, cosq, sinq, TOK)

                _rel(mmp)
                _cut(3)

                # ---------------- attention ----------------
                p_out = es.enter_context(tc.tile_pool(name="p_out", bufs=1,
                                                      side="right"))
                outT = [p_out.tile([128, TOK], F32, name=f"outT{i}")
                        for i in range(CT)]
                p_at = _new(name="p_at", bufs=1, side="right")
                attnT = [p_at.tile([128, TOK], F32R, name=f"attnT{i}")
                         for i in range(CT)]
                with tc.tile_pool(name="attsb", bufs=12) as asb, \
                     tc.tile_pool(name="scps", bufs=4, space="PSUM") as scps, \
                     tc.tile_pool(name="atps", bufs=2, space="PSUM") as atps:
                    for ci, (kvlen, mask) in enumerate(((512, mlo), (T, mhi))):
                        qsl = slice(ci * CHK, (ci + 1) * CHK)
                        for h in range(H):
                            mt, po = h // 2, (h % 2) * 64
                            ktiles = kvlen // 128
                            probs = []
                            for kt in range(ktiles):
                                pt = scps.tile([128, CHK], F32, name="scp", tag="sc")
                                nc.tensor.matmul(
                                    pt[:],
                                    lhsT=rkT[mt][po:po + 64,
                                                 kt * 128:(kt + 1) * 128],
                                    rhs=rqT[mt][po:po + 64, qsl],
                                    start=True, stop=True)
                                ms = asb.tile([128, CHK], F32, name="ms",
                                              tag="ms", bufs=6)
                                nc.vector.tensor_tensor(
                                    ms[:], pt[:],
                                    mask[:, kt * CHK:(kt + 1) * CHK],
                                    op=ALU.add)
                                pr = asb.tile([128, CHK], F32R, name="pr", tag="pr",
                                              bufs=10)
                                nc.scalar.activation(pr[:], ms[:], AF.Exp)
                                probs.append(pr)
                            ap = atps.tile([65, CHK], F32, name="atp", tag="at")
                            for kt in range(ktiles):
                                nc.tensor.matmul(
                                    ap[:], lhsT=vsb[kt][:, h * 65:(h + 1) * 65],
                                    rhs=probs[kt][:],
                                    start=(kt == 0), stop=(kt == ktiles - 1))
                            rcp = asb.tile([1, CHK], F32, name="rcp", tag="rcp",
                                           bufs=3)
                            nc.vector.reciprocal(rcp[:], ap[64:65, :])
                            nc.vector.tensor_scalar(rcp[:], rcp[:], sca[0:1, 3:4],
                                                    None, op0=ALU.mult)
                            rcb = asb.tile([64, CHK], F32, name="rcb", tag="rcb",
                                           bufs=3)
                            nc.gpsimd.partition_broadcast(rcb[:], rcp[:])
                            nc.vector.tensor_tensor(
                                attnT[mt][po:po + 64, qsl], ap[0:64, :], rcb[:],
                                op=ALU.mult)

                _rel(p_kv)       # free vsb
                _rel(p_qk)       # free rq/rk
                _cut(4)

                # ---------------- Wo + residual ----------------
                mmp = _new(name="mmp2", bufs=7, space="PSUM",
                                         side="right")
                p_outr = _new(name="p_outr", bufs=1)
                outTr = [p_outr.tile([128, TOK], F32R, name=f"outTr{i}")
                         for i in range(CT)]
                with tc.tile_pool(name="wop", bufs=3) as wop:
                    for grp in range(2):
                        pts = [mmp.tile([128, 512], F32, name="wops", tag="mm")
                               for _ in range(3)]
                        for k in range(CT):
                            wt = wop.tile([128, 384], F32R, name="wot", tag="wot")
                            nc.sync.dma_start(
                                wt[:], wo_d.ap()[k * 128:(k + 1) * 128,
                                                 grp * 384:(grp + 1) * 384])
                            for mi in range(3):
                                nc.tensor.matmul(
                                    pts[mi][:], lhsT=wt[:, mi * 128:(mi + 1) * 128],
                                    rhs=attnT[k][:],
                                    start=(k == 0), stop=(k == CT - 1))
                        for mi in range(3):
                            m = grp * 3 + mi
                            nc.vector.tensor_tensor(outT[m][:], pts[mi][:],
                                                    xqT[m][:], op=ALU.add)
                            nc.scalar.copy(outTr[m][:], outT[m][:])

                _rel(p_at)       # free attnT (top of right stack)

                # ---------------- sum(out^2) -> AR2 ----------------
                with tc.tile_pool(name="sqp2", bufs=2) as sqp:
                    for ct in range(CT):
                        sq = sqp.tile([128, TOK], F32, name="sq2", tag="sq")
                        nc.vector.tensor_tensor(sq[:], outT[ct][:], outT[ct][:],
                                                op=ALU.mult)
                        nc.vector.reduce_sum(ss2_sb[:, ct:ct + 1], sq[:], axis=AX)
                nc.vector.tensor_tensor(ss2_sb[:, 6:7], ss2_sb[:, 0:1],
                                        ss2_sb[:, 1:2], op=ALU.add)
                for ct in range(2, CT):
                    nc.vector.tensor_tensor(ss2_sb[:, 6:7], ss2_sb[:, 6:7],
                                            ss2_sb[:, ct:ct + 1], op=ALU.add)
                ssp2 = s1ps.tile([1, 1], F32, name="ssp2", tag="ss")
                nc.tensor.matmul(ssp2[:], lhsT=ss2_sb[:, 6:7], rhs=ones_col[:],
                                 start=True, stop=True)
                nc.vector.tensor_copy(scb[:, 7:8], ssp2[:])
                nc.sync.dma_start(ar2_in[:], scb[:, 7:8])
                _rel(s1ps)
                if collectives:
                    nc.gpsimd.collective_compute(
                        "AllReduce", ALU.add, replica_groups=[list(range(NC))],
                        ins=[ar2_in.opt()], outs=[ar2_out.opt()])
                else:
                    nc.sync.dma_start(ar2_out[:], ar2_in[:])
                _cut(5)

                # ---------------- FF ----------------
                w1p = _new(name="w1p", bufs=7)
                p_u = _new(name="p_u", bufs=1, side="right")
                uraw = _new(name="uraw", bufs=1)
                uT = [p_u.tile([128, TOK], FDT, name=f"uT{i}") for i in range(FT)]

                nc.sync.dma_start(scb[:, 0:1], ar2_out[:])
                nc.vector.tensor_scalar(scb[:, 1:2], scb[:, 0:1], 1.0 / NUMEL, EPS,
                                        op0=ALU.mult, op1=ALU.add)
                nc.scalar.sqrt(scb[:, 2:3], scb[:, 1:2])
                nc.vector.reciprocal(scb[:, 3:4], scb[:, 2:3])   # s2
                nc.gpsimd.partition_broadcast(s2b[:], scb[:, 3:4])
                for grp in range(6):
                    pts = [mmp.tile([128, 512], F32, name="ups", tag="mm")
                           for _ in range(4)]
                    for k in range(CT):
                        pan = w1p.tile([128, 512], F32R, name="w1pan", tag="w1pan",
                                       bufs=3)
                        nc.sync.dma_start(
                            pan[:], w1_d.ap()[k * 128:(k + 1) * 128,
                                              grp * 512:(grp + 1) * 512])
                        nc.vector.tensor_scalar(pan[:], pan[:], g2c[:, k:k + 1],
                                                None, op0=ALU.mult)
                        for mi in range(4):
                            nc.tensor.matmul(
                                pts[mi][:], lhsT=pan[:, mi * 128:(mi + 1) * 128],
                                rhs=outTr[k][:], start=(k == 0), stop=(k == CT - 1))
                    for mi in range(4):
                        m = grp * 4 + mi
                        ar = uraw.tile([128, 512], F32, name="uraw",
                                       tag="uraw", bufs=8)
                        nc.scalar.copy(ar[:], pts[mi][:])
                        nc.vector.tensor_scalar(uT[m][:], ar[:], s2b[:],
                                                b1c[:, m:m + 1],
                                                op0=ALU.mult, op1=ALU.add)

                _rel(uraw)
                _rel(w1p)
                _rel(p_outr)     # free outTr
                _cut(6)

                wstr = _new(name="wstr", bufs=10)
                p_g = _new(name="p_g", bufs=1)
                gT = [p_g.tile([128, TOK], FDT, name=f"gT{i}") for i in range(HT)]
                with tc.tile_pool(name="wxsb", bufs=1) as wxsb:
                    for grp in range(4):
                        wxs = []
                        pts = [mmp.tile([128, 512], F32, name="wxps", tag="mm")
                               for _ in range(4)]
                        for k in range(FT):
                            wt = wstr.tile([128, 512], FDT, name="wwt", tag="wst")
                            nc.sync.dma_start(
                                wt[:], ww_d.ap()[k * 128:(k + 1) * 128,
                                                 grp * 512:(grp + 1) * 512])
                            for mi in range(4):
                                nc.tensor.matmul(
                                    pts[mi][:], lhsT=wt[:, mi * 128:(mi + 1) * 128],
                                    rhs=uT[k][:],
                                    start=(k == 0), stop=(k == FT - 1))
                        for mi in range(4):
                            m = grp * 4 + mi
                            wx = wxsb.tile([128, 512], F32, name="wxs", tag="wxs",
                                           bufs=5)
                            nc.scalar.activation(wx[:], pts[mi][:],
                                                 AF.Identity,
                                                 bias=bwc[:, m:m + 1])
                            wxs.append(wx)
                        pts = [mmp.tile([128, 512], F32, name="vxps", tag="mm")
                               for _ in range(4)]
                        for k in range(FT):
                            wt = wstr.tile([128, 512], FDT, name="wgt", tag="wst")
                            nc.sync.dma_start(
                                wt[:], wg_d.ap()[k * 128:(k + 1) * 128,
                                                 grp * 512:(grp + 1) * 512])
                            for mi in range(4):
                                nc.tensor.matmul(
                                    pts[mi][:], lhsT=wt[:, mi * 128:(mi + 1) * 128],
                                    rhs=uT[k][:],
                                    start=(k == 0), stop=(k == FT - 1))
                        for mi in range(4):
                            m = grp * 4 + mi
                            sil = wxsb.tile([128, 512], F32, name="sil", tag="sil",
                                            bufs=3)
                            nc.scalar.activation(sil[:], pts[mi][:], AF.Silu,
                                                 bias=bgc[:, m:m + 1])
                            nc.vector.tensor_tensor(gT[m][:], wxs[mi][:], sil[:],
                                                    op=ALU.mult)

                _rel(p_u)        # free uT
                _cut(7)
                p_f1 = _new(name="p_f1", bufs=1, side="right")
                ff1T = [p_f1.tile([128, TOK], F32R, name=f"ff1T{i}")
                        for i in range(FT)]

                for grp in range(6):
                    pts = [mmp.tile([128, 512], F32, name="f1ps", tag="mm")
                           for _ in range(4)]
                    for k in range(HT):
                        wt = wstr.tile([128, 512], FDT, name="wdt", tag="wst")
                        nc.sync.dma_start(
                            wt[:], wd_d.ap()[k * 128:(k + 1) * 128,
                                             grp * 512:(grp + 1) * 512])
                        for mi in range(4):
                            nc.tensor.matmul(
                                pts[mi][:], lhsT=wt[:, mi * 128:(mi + 1) * 128],
                                rhs=gT[k][:],
                                start=(k == 0), stop=(k == HT - 1))
                    for mi in range(4):
                        m = grp * 4 + mi
                        nc.scalar.activation(ff1T[m][:], pts[mi][:],
                                             AF.Identity,
                                             bias=bdc[:, m:m + 1])

                _rel(p_g)       # free gT
                _cut(8)

                with tc.tile_pool(name="yout", bufs=3) as yout:
                    for grp in range(2):
                        pts = [mmp.tile([128, 512], F32, name="yps", tag="mm")
                               for _ in range(3)]
                        for k in range(FT):
                            wt = wstr.tile([128, 384], F32R, name="w2t", tag="wst")
                            nc.sync.dma_start(
                                wt[:], w2_d.ap()[k * 128:(k + 1) * 128,
                                                 grp * 384:(grp + 1) * 384])
                            for mi in range(3):
                                nc.tensor.matmul(
                                    pts[mi][:], lhsT=wt[:, mi * 128:(mi + 1) * 128],
                                    rhs=ff1T[k][:],
                                    start=(k == 0), stop=(k == FT - 1))
                        for mi in range(3):
                            m = grp * 3 + mi
                            yt = yout.tile([128, 512], F32, name="yt", tag="yt")
                            nc.vector.tensor_tensor(yt[:], pts[mi][:], outT[m][:],
                                                    op=ALU.add)
                            nc.vector.tensor_scalar(yt[:], yt[:], b2c[:, m:m + 1],
                                                    None, op0=ALU.add)
                            nc.sync.dma_start(y_d.ap()[m * 128:(m + 1) * 128, :],
                                              yt[:])

            except _Stop:
                pass
            for _p in reversed(_open):
                _p.release()
            es.close()

    nc.compile()
    return nc


FF_BF16 = False


def _host_inputs(x, Wq, Wk, Wv, Wo, g1, g2, W1, b1, Ww, bw, Wg, bg, Wd, bd,
                 W2, b2, ff_bf16=None):
    import ml_dtypes
    if ff_bf16 is None:
        ff_bf16 = FF_BF16
    fdt = ml_dtypes.bfloat16 if ff_bf16 else np.float32
    f32 = np.float32
    cos, sin = _rope_tables_np()
    cosf = np.ascontiguousarray(np.tile(cos.T, (2, 1)), dtype=f32)
    sinf = np.ascontiguousarray(np.tile(sin.T, (2, 1)), dtype=f32)

    def cols(v, n):
        return np.ascontiguousarray(np.asarray(v, f32).reshape(n, 128).T)

    R = np.zeros((S, S), f32)
    for i in range(S // 2):
        R[2 * i + 1, 2 * i] = -1.0
        R[2 * i, 2 * i + 1] = 1.0
    rmat = np.zeros((128, 128), f32)
    rmat[:64, :64] = R
    rmat[64:, 64:] = R

    shared = {
        "wq": np.ascontiguousarray(
            np.transpose(np.asarray(Wq, f32), (1, 0, 2)).reshape(C, C)),
        "wk": np.ascontiguousarray(
            np.transpose(np.asarray(Wk, f32), (1, 0, 2)).reshape(C, C)),
        "wv": np.ascontiguousarray(
            np.transpose(np.asarray(Wv, f32), (1, 0, 2)).reshape(C, C)),
        "wo": np.ascontiguousarray(np.asarray(Wo, f32)),
        "w1": np.ascontiguousarray(np.asarray(W1, f32)),
        "ww": np.ascontiguousarray(np.asarray(Ww, f32).astype(fdt)),
        "wg": np.ascontiguousarray(np.asarray(Wg, f32).astype(fdt)),
        "wd": np.ascontiguousarray(np.asarray(Wd, f32).astype(fdt)),
        "w2": np.ascontiguousarray(np.asarray(W2, f32)),
        "g1c": cols(g1, CT), "g2c": cols(g2, CT),
        "b1c": cols(b1, FT), "bwc": cols(bw, HT), "bgc": cols(bg, HT),
        "bdc": cols(bd, FT), "b2c": cols(b2, CT),
        "cosf": cosf, "sinf": sinf, "rmat": rmat,
    }

    x = np.asarray(x, f32)
    in_maps = []
    for core in range(NC):
        b, half = divmod(core, 2)
        idx = _own_idx(half)
        m = dict(shared)
        m["xb"] = np.ascontiguousarray(x[b])
        m["xq"] = np.ascontiguousarray(x[b][idx])
        m["cosq"] = np.ascontiguousarray(cosf[:, idx])
        m["sinq"] = np.ascontiguousarray(sinf[:, idx])
        (l0, l1), (h0, h1) = _chunks_for(half)
        m["mlo"] = np.where(
            np.arange(512)[:, None] <= np.arange(l0, l1)[None, :],
            0.0, -1e30).astype(f32)
        m["mhi"] = np.where(
            np.arange(T)[:, None] <= np.arange(h0, h1)[None, :],
            0.0, -1e30).astype(f32)
        in_maps.append(m)
    return in_maps


def kernel(**inputs):
    from concourse import bass_utils
    if "nc" not in _cache:
        _cache["nc"] = _build_program(ff_bf16=FF_BF16)
    nc = _cache["nc"]
    in_maps = _host_inputs(**inputs)
    res = bass_utils.run_bass_kernel_spmd(nc, in_maps,
                                          core_ids=list(range(NC)))
    y = np.empty((B, T, C), np.float32)
    for core in range(NC):
        b, half = divmod(core, 2)
        y[b, _own_idx(half), :] = res.results[core]["yT"].T
    return y

